# revision 4
# baseline (speedup 1.0000x reference)
"""Trainium2 Bass kernel for nn_HausdorffDistance_28406913696124.

Math (reference):
    px = (prob_map[0].ravel() >= 0.5)                 # [N], N = 100*100
    py = (gt_map.ravel()   >= 0.5)                    # [N]
    D[i,j] = euclid dist between grid points i, j     # [N, N] constant!
    loss   = mean_i | px_i * mean_j D[i,j] - (D @ py)_i / N |

Key structure (same math as the previous revision):
  * rowmean_i = mean_j D[i,j] is a pure geometric constant -> host table.
  * (D @ py) is a 2D correlation of the binary mask PY with the 199x199
    kernel K(u,v) = sqrt(u^2 + v^2).  A displacement-count weighted SVD
    makes K rank-1 separable to ~4e-4 final relative error (tol 2e-2):
    two chained 100x100 matmuls against constant Toeplitz tables.
  * All scalar factors are folded into the constant tables so a single
    abs-row-reduce + cross-partition add yields the loss directly.
  * term1 is folded in by pre-writing t1n = px * (-1e-8 * rowsum) into the
    stage-2 PSUM accumulation bank before the matmul (start=False).

This revision removes the absolute value EXACTLY and restructures the
schedule around the TimelineSim cost model (each DMA pays a fixed
~1.3us issue->transfer pipeline plus a 900ns completion-semaphore
latency; every cross-engine hop costs ~40-130ns):

  * De-abs identity: term2_i = (subset-sum of non-negative D over
    py)/N <= rowmean_i always, so |t1-t2| == (2px-1)(t1-t2) for ANY
    input and loss*N^2 = sum(px*rowsum) + sum((1-2px)*T2raw).  Stage 2
    therefore contracts the sign mask sigma = 0.5-px (ready right
    after the binarize) against a CONSTANT transposed-f table, running
    back-to-back with stage 1 on the PE; the finale is a same-engine
    DVE pair (G-copy, then fused multiply + row-accumulate against the
    sigma-stage PSUM bank) plus one Pool cross-partition add over a
    [100,2] partials tile that also carries the term1 partial sums
    (computed in the DVE's idle shadow).
  * ONE input DMA on the SP HWDGE ring (five f16 tables, 1000B/row
    descriptors -- over the <512B 2x DMA latency threshold), hoisted
    ABOVE the prologue all-engine barrier by post-compile surgery so
    the fixed pipeline overlaps the barrier.  Its completion sem fires
    ~2.4us in, long after the hoisted ~60ns tile-sem clear, so
    dirty-device entry stays safe on any hardware timing.
  * Everything is f16 (gt/prob threshold flips from f32->f16 rounding
    move the loss by ~0.1%, well inside the 2e-2 gate).
  * Same-engine DVE waits are stripped by surgery (Tile guards the
    g16 RAW through a ~260ns sem round-trip; the DVE executes its
    queue in order, so program order already provides the ordering).
  * Epilogue surgery: only Pool's round-1 barrier gather survives (the
    other engines' release waits AND the matching release updates are
    stripped so the barrier sems still return to zero for the next run);
    the redundant end-of-program sem-range clear is dropped (the
    program's own start-of-run clear covers a superset, 140-189 vs
    156-169); the writeback-completion wait (+16 on the prep's SWDGE
    lane sem, rewired from the descriptor-baked sem as before) moves to
    the very LAST Pool instruction so the post-gather drain/evsem tail
    executes inside the 900ns completion window instead of after it.

Single core, no collectives: the whole problem is ~0.5 MFLOP, so the
8-core AllReduce floor (~5us) dwarfs any compute sharding gain.
"""

import os
import sys

import numpy as np

# Prefer the .axon_site concourse (the stack this kernel was validated
# against); /opt/trn_rl_repo is the fallback.
for _p in ("/opt/trn_rl_repo", "/root/.axon_site/_ro/trn_rl_repo"):
    if os.path.isdir(_p):
        sys.path.insert(0, _p)

H = 100
N = H * H
S_A = 1e-4    # scale folded into the stage-1 moving table
S_B = 1e-4    # scale folded into the stage-2 stationary table


def _host_constants():
    """Geometry-only constant tables (input independent)."""
    idx = np.arange(H)

    # Displacement-count weighted SVD of K(u,v) = sqrt(u^2+v^2) on
    # [-99,99]^2: weight sqrt(100-|u|) per axis makes the truncation error
    # small exactly where displacements are frequent.  Rank 1 reproduces
    # the final scalar to ~4e-4 relative (tolerance is 2e-2).
    u = np.arange(-(H - 1), H)
    K = np.sqrt((u[:, None] ** 2 + u[None, :] ** 2).astype(np.float64))
    wu = np.sqrt((H - np.abs(u)).astype(np.float64))
    Uw, S, Vtw = np.linalg.svd(wu[:, None] * K * wu[None, :])
    ffac = (Uw[:, 0] * np.sqrt(S[0])) / wu    # [199]
    gfac = (Vtw[0, :] * np.sqrt(S[0])) / wu   # [199]

    # Toeplitz tables: offs[a, r] = (r - a) + 99.  ft2[r, a] = f(r - a) is
    # the stage-2 stationary for the SIGN-mask contraction (see module
    # docstring); its extra factor 2 absorbs the 1/2 in sigma = 0.5 - px.
    offs = (idx[None, :] - idx[:, None]) + (H - 1)
    m1 = (gfac[offs] * S_A).astype(np.float16)          # [100, 100]
    ft2 = (ffac[offs].T * (2 * S_B)).astype(np.float16)  # [100, 100]

    # rowsum[r,c] = sum_j D[i,j] (i = r*100+c), f64-exact, scaled by +1e-8:
    # the de-abs'd loss ADDS px*rowsum/N^2 (see docstring).
    absdiff = np.abs(idx[:, None] - idx[None, :])
    q = np.sqrt((idx[:, None] ** 2 + idx[None, :] ** 2).astype(np.float64))
    cnt = np.zeros((H, H))
    np.add.at(cnt, (idx[:, None], absdiff), 1.0)
    rm16 = ((cnt @ q @ cnt.T) * (S_A * S_B)).astype(np.float16)

    return m1, ft2, rm16


def _build_module():
    import concourse.bacc as bacc
    import concourse.mybir as mybir
    import concourse.tile as tile

    f32 = mybir.dt.float32
    f16 = mybir.dt.float16

    nc = bacc.Bacc(
        "TRN2",
        target_bir_lowering=False,
        debug=False,
        enable_asserts=False,
        num_devices=1,
    )

    W1 = 5 * H      # pack1: gtT | m1 | prob | ft2 | rm   (f16)
    pack1_d = nc.dram_tensor("pack1", [H, W1], f16, kind="ExternalInput")
    # kv_writeback stores a full 128-partition column; slot 0 is the answer.
    out_d = nc.dram_tensor("out", [1, 128], f32, kind="ExternalOutput")

    dma_sem = nc.alloc_semaphore("out_wb_done")
    out_val = nc.alloc_sbuf_tensor("out_val", [128, 1], f32)

    # Unrelated NEFFs (the tiny XLA jit programs the host runs in the same
    # process) leave semaphores dirty, and this program's waits assume they
    # start at zero.  Two clears: the tile-context sem range [156,190) is
    # hoisted by surgery to Activation's FIRST instruction, so it is
    # guaranteed (by ~2us of DMA pipeline) to precede the hoisted input
    # DMA's completion update on any hardware timing; the low range --
    # which contains the prologue barrier's own gather/release sems --
    # stays at its natural post-barrier position where clearing it is
    # harmless.
    nc.scalar.sem_clear(range(156, 190))
    nc.scalar.sem_clear(range(140, 156))

    with tile.TileContext(nc) as tc:
        with (
            tc.tile_pool(name="sb", bufs=1) as sb,
            tc.tile_pool(name="ps_g", bufs=1, space="PSUM") as ps_g,
            tc.tile_pool(name="ps_acc", bufs=1, space="PSUM") as ps_acc,
        ):
            # ---- load: ONE SP-HWDGE DMA (1000B/row descriptors, well
            # over the <512B 2x DMA latency threshold), hoisted pre-barrier
            # by surgery. ----
            pk1 = sb.tile([H, W1], f16)
            nc.sync.dma_start(pk1[:], pack1_d[:])

            # ---- output writeback, prepared early: the SWDGE descriptors
            # are generated during Pool's idle window, so after the final
            # reduce only the trigger (SEQ op + transfer + completion sem)
            # is left -- skipping the ~1.3us HWDGE+DGE output pipeline.
            # out_val is a RAW sbuf tensor (not a tile): Tile must not see
            # the prep's deferred read, or it inserts a WAR wait on the
            # transfer completion in front of the reduce (deadlock).  The
            # read-after-write ordering is enforced at the trigger instead
            # (signals_writable on rowsums, which the Pool reduce consumes
            # in-order right before the trigger). ----
            out_ap = out_val.ap()
            wb_idx = sb.tile([128, 1], mybir.dt.int32)
            nc.vector.memset(out_ap, 0.0)
            nc.vector.memset(wb_idx[:], 0)
            nc.gpsimd.kv_writeback(
                out_d[:].rearrange("b (d o n) -> b d o n", o=1, n=1),
                out_ap.rearrange("d (o b n) -> d o b n", b=1, n=1),
                wb_idx[:],
                prepare_only=True,
                sem=dma_sem,
            )

            gtT = pk1[:, 0:H]                 # [100,100] f16 (raw gt values)
            m1 = pk1[:, H:2 * H]              # [100,100] f16
            prob = pk1[:, 2 * H:3 * H]        # [100,100] f16
            ft2 = pk1[:, 3 * H:4 * H]         # [100,100] f16
            rm = pk1[:, 4 * H:5 * H]          # [100,100] f16

            # partials[:,0] <- sum_c G*V (the sign-weighted term2), and
            # partials[:,1] <- sum_c px*rowsum/N^2 (term1).  Zeroed up
            # front in case the DVE accumulate-output adds to prior
            # contents.
            partials = sb.tile([H, 2], f32)
            nc.vector.memset(partials[:], 0.0)

            # ---- binarize the transposed mask (stage-1 stationary) ------
            pyt = sb.tile([H, H], f16)
            nc.vector.tensor_scalar(
                pyt[:], gtT, 0.5, None, mybir.AluOpType.is_ge
            )

            # ---- sigma = 0.5 - px, the exact de-abs sign mask (x2 folded
            # into ft2).  |t1-t2| == (2px-1)(t1-t2) holds EXACTLY: t2_i is
            # a subset-sum of non-negative D over py (/N), so t2 <= rowmean
            # wherever px=1, and t1=0 <= t2 wherever px=0. ----------------
            sigma = sb.tile([H, H], f16)
            nc.vector.tensor_scalar(
                sigma[:], prob, 0.5, 0.5,
                mybir.AluOpType.is_lt, mybir.AluOpType.subtract,
            )

            # ---- term1 partials: px * (rowsum/N^2), row-accumulated -----
            scrap2 = sb.tile([H, H], f16)
            nc.vector.scalar_tensor_tensor(
                scrap2[:],
                prob,
                0.5,
                rm,
                op0=mybir.AluOpType.is_ge,
                op1=mybir.AluOpType.mult,
                accum_out=partials[:, 1:2],
            )

            # ---- stage 1: G[a, c] = sum_b pyt[b,a] * m1[b, c] ------------
            g_ps = ps_g.tile([H, H], f32)
            nc.tensor.matmul(g_ps[:], pyt[:], m1, start=True, stop=True)

            # ---- stage 2 on the SIGN mask (constant stationary; runs
            # back-to-back with stage 1 on the PE -- no PSUM->SBUF copy of
            # G is needed any more): V[a,c] = sum_r f(r-a)*2*sigma[r,c] ---
            v_ps = ps_acc.tile([H, H], f32)
            nc.tensor.matmul(v_ps[:], ft2, sigma[:], start=True, stop=True)

            # ---- final contraction: partials[:,0] = sum_c G[a,c]*V[a,c].
            # DVE ops may read only ONE PSUM operand (verifier), so G is
            # first copied to SBUF -- but copy and fused multiply+row-
            # accumulate run back-to-back on DVE with no cross-engine hop,
            # unlike the old G-copy -> PE matmul -> DVE abs-reduce chain. -
            g16 = sb.tile([H, H], f16)
            nc.vector.tensor_scalar(
                g16[:], g_ps[:], 1.0, None, mybir.AluOpType.mult
            )
            scrap1 = sb.tile([H, H], f32)
            nc.vector.scalar_tensor_tensor(
                scrap1[:],
                v_ps[:],
                1.0,
                g16[:],
                op0=mybir.AluOpType.mult,
                op1=mybir.AluOpType.mult,
                accum_out=partials[:, 0:1],
            )
            nc.gpsimd.tensor_reduce(
                out_ap[0:1, 0:1],
                partials[:],
                axis=mybir.AxisListType.XYZWC,
                op=mybir.AluOpType.add,
            )
            # Fire the pre-built writeback.  signals_writable=[partials]
            # gives the trigger a WAR dep on the Pool reduce's read, and
            # Pool executes in order, so the scalar is committed before
            # the transfer starts.
            nc.gpsimd.trigger_dma(signals_writable=[partials[:]])

    nc.compile()
    _surgery(nc)
    return nc


def _surgery(nc):
    """Post-compile BIR surgery (compile regenerates event-semaphore waits,
    so this must run on the final BIR).  See module docstring."""
    import concourse.mybir as mybir

    blocks = nc.m.functions[0].blocks
    main_bb = blocks[0]
    body_bbs = [
        bb for bb in blocks
        if bb.name.startswith("tile_context") and not bb.name.endswith("_end")
    ]
    end_bbs = [bb for bb in blocks if bb.name.endswith("_end")]
    assert len(body_bbs) == 1 and len(end_bbs) == 1, [b.name for b in blocks]
    body_bb, end_bb = body_bbs[0], end_bbs[0]

    E = mybir.EngineType

    # ---- S1: rewire the kv-writeback prep's completion update ----------
    # Tile schedules the prep on a dedicated DMASW lane and makes the
    # epilogue wait for that lane's completion sem (+16), but the
    # descriptor-baked completion sem is the one passed as `sem=`.
    # Rewrite the prep's completion update to target the (otherwise
    # orphaned) DMASW lane sem so the epilogue wait is satisfied by the
    # actual transfer completion.
    updated_ids = set()
    waited = {}
    prep = None
    for bb in blocks:
        for inst in bb.instructions:
            if type(inst).__name__ == "InstKVWritebackAnt":
                prep = inst
            si = inst.sync_info
            if si is None:
                continue
            for u in si.on_update:
                updated_ids.add(u.id)
            for w in si.on_wait:
                if w.ant_name and w.ant_name.startswith("DMASW"):
                    waited[w.id] = w.ant_name
    orphans = {i: n for i, n in waited.items() if i not in updated_ids}
    assert prep is not None and len(orphans) == 1, (prep, orphans)
    ((oid, oname),) = orphans.items()
    si = prep.sync_info
    old = list(si.on_update)
    si.on_update = [
        mybir.SyncUpdate(
            sync_type="semaphore",
            id=oid,
            ant_name=oname,
            update_mode="sem-add-imm",
            update_value=16,
            update_reg=None,
        )
    ] + old[1:]

    # ---- S2a: strip same-engine DVE waits from DVE body instructions ---
    # Tile guards the g16 RAW (DVE copy -> DVE fused multiply) through the
    # full semaphore round-trip (~260ns); the DVE engine executes its
    # queue in order, so program order already provides that ordering.
    for inst in body_bb.instructions:
        if inst.engine != E.DVE:
            continue
        si_d = inst.sync_info
        if si_d is None:
            continue
        kept_d = [
            w for w in si_d.on_wait
            if not (w.ant_name or "").startswith("DVE_")
        ]
        if len(kept_d) != len(list(si_d.on_wait)):
            si_d.on_wait = kept_d

    # ---- S2: strip guards from body-block Pool event sems --------------
    # Tile guards the Pool Q7 library reload (for the XYZWC reduce) behind
    # "SWDGE queue drained" -- circular with a prepared-untriggered DMA --
    # and the trigger behind a SEQ-level DVE guard that duplicates the
    # engine-level data waits.  Both are unnecessary here: every data
    # dependency lives on the consuming engine instruction itself (the
    # reduce waits for DVE, the trigger's DMA stage waits for the Pool
    # engine-op count), and Pool's engine executes in order.  Stripping
    # them lets the trigger's 132ns SEQ ISA pre-execute.
    for inst in body_bb.instructions:
        if type(inst).__name__ != "InstEventSemaphore":
            continue
        si2 = inst.sync_info
        if si2 is None:
            continue
        if inst.engine == E.Pool:
            si2.on_wait = []
        else:
            kept = [w for w in si2.on_wait if w.id != oid]
            if len(kept) != len(list(si2.on_wait)):
                si2.on_wait = kept

    # ---- S3: hoist the sem clear + both input DMAs above the prologue
    # barrier.  Target per-engine prologue order:
    #   SP:  Drain, DMACopy(pack1), EventSemaphore(barrier)
    #   ACT: Drain, ISA(sem clear), DMACopy(pack2), EventSemaphore(barrier)
    # The DMA completion sems fire ~2.3us in, far after the ~90ns clear,
    # so dirty-entry safety matches the unhoisted program.
    def _pop(bb, pred):
        for i, inst in enumerate(bb.instructions):
            if pred(inst):
                del bb.instructions[i]
                return inst
        raise AssertionError("instruction not found")

    def _index(bb, pred):
        for i, inst in enumerate(bb.instructions):
            if pred(inst):
                return i
        raise AssertionError("instruction not found")

    # (The program's sem-range clear [140,189] stays at its natural
    # post-barrier position: it covers the barrier gather/release sems
    # themselves, so running it mid-barrier could wipe pending gather
    # updates on hardware.  At ~620ns it still precedes both the first
    # completion-sem update (~2.4us) and the first data-consumer wait
    # activation (~900ns), same safety story as the previous revision.)
    if os.environ.get("K2_NO_HOIST") != "1":
        # the tile-sem clear goes FIRST in the Activation stream (before
        # even its Drain): it must beat the hoisted DMA's completion-sem
        # update by a margin that holds on real hardware, not just in the
        # cost model.
        hi_clear = _pop(
            main_bb,
            lambda x: type(x).__name__ == "InstISA"
            and x.ant_dict.get("range_first") == 156,
        )
        i_act = _index(
            main_bb,
            lambda x: type(x).__name__ == "InstDrain"
            and x.engine == E.Activation,
        )
        main_bb.instructions.insert(i_act, hi_clear)
        sp_dma = _pop(
            body_bb,
            lambda x: type(x).__name__ == "InstDMACopy" and x.engine == E.SP,
        )
        si3 = sp_dma.sync_info
        assert si3 is None or not list(si3.on_wait), sp_dma.name
        # ahead even of SP's program-start Drain: the DMACopy is pure
        # SEQ->HWDGE work with no waits, and PJRT serializes executions, so
        # there is no stale engine state it could race.
        i_sp = _index(
            main_bb,
            lambda x: type(x).__name__ == "InstDrain" and x.engine == E.SP,
        )
        main_bb.instructions.insert(i_sp, sp_dma)

    # ---- S4: epilogue restructure.  Keep: per-engine round-1 Drains
    # (gather updates) + Pool's round-1 gather wait.  Strip: the other
    # engines' release waits AND all release updates (so gather/release
    # both return to 0 for the next run's prologue barrier), every
    # DMASW/Pool_sequencer wait Tile placed, and the entire round-2
    # barrier.  Drop the end-of-program sem-range clear (the start-of-run
    # clear covers a superset).  Finally, put the writeback-completion
    # wait (orphan lane >= 16) on the very LAST Pool instruction so the
    # drain/evsem tail runs inside the DMA completion window.
    def _is_release(w_or_u):
        return "release" in (w_or_u.ant_name or "")

    def _is_gather(w_or_u):
        return (w_or_u.ant_name or "").endswith("gather")

    if os.environ.get("K2_LITE_EPI") == "1":
        # Baseline-style epilogue: DMASW wait moves onto the Pool gather,
        # Pool_sequencer waits stripped, round-2 (post-ISA) neutered.
        pool_gather = None
        for inst in end_bb.instructions:
            si4 = inst.sync_info
            if si4 is None:
                continue
            if (
                pool_gather is None
                and inst.engine == E.Pool
                and any(_is_gather(w) for w in si4.on_wait)
            ):
                pool_gather = inst
            si4.on_wait = [
                w for w in si4.on_wait
                if w.id != oid
                and not (w.ant_name or "").startswith("DMASW")
                and not (w.ant_name or "").startswith("Pool_sequencer")
            ]
        assert pool_gather is not None
        si5 = pool_gather.sync_info
        si5.on_wait = list(si5.on_wait) + [
            mybir.SyncWait(
                sync_type="semaphore", id=oid, ant_name=oname,
                wait_mode="sem-ge-imm", wait_value=16, wait_reg=None,
            )
        ]
        seen_isa = False
        for inst in end_bb.instructions:
            tn = type(inst).__name__
            if tn == "InstISA":
                seen_isa = True
                continue
            if seen_isa and tn in ("InstDrain", "InstEventSemaphore"):
                si6 = inst.sync_info
                if si6 is not None:
                    si6.on_wait = []
                    si6.on_update = []
        return

    # drop the end-of-program sem clear entirely
    _pop(end_bb, lambda x: type(x).__name__ == "InstISA")

    _seen_gather_updates = set()
    pool_gather_seen = False
    for inst in end_bb.instructions:
        si4 = inst.sync_info
        if si4 is None:
            continue
        waits = list(si4.on_wait)
        ups = list(si4.on_update)
        new_waits = []
        for w in waits:
            nm = w.ant_name or ""
            if w.id == oid or nm.startswith("DMASW"):
                continue  # completion wait re-added on the last Pool inst
            if nm.startswith("Pool_sequencer"):
                continue  # rides the 900ns DMA overhead path
            if _is_release(w):
                if inst.engine == E.Pool and not pool_gather_seen:
                    new_waits.append(w)  # (gather wait handled below)
                continue
            if _is_gather(w):
                if inst.engine == E.Pool and not pool_gather_seen:
                    pool_gather_seen = True
                    new_waits.append(w)
                continue
            new_waits.append(w)
        new_ups = []
        for u in ups:
            if _is_release(u):
                continue
            if _is_gather(u):
                # keep round-1 drain gather+1 updates (feed Pool's gather);
                # round-2 comes after the Pool gather was seen AND is a
                # second gather-update from the same engine -- strip by
                # position: only the FIRST gather update per engine stays.
                key = ("g", inst.engine)
                if key in _seen_gather_updates:
                    continue
                _seen_gather_updates.add(key)
            new_ups.append(u)
        si4.on_wait = new_waits
        si4.on_update = new_ups

    # final Pool instruction carries the transfer-completion wait
    if os.environ.get("K2_NO_FINAL_WAIT") == "1":
        return
    last_pool = None
    for inst in end_bb.instructions:
        if inst.engine == E.Pool:
            last_pool = inst
    assert last_pool is not None
    si5 = last_pool.sync_info
    assert si5 is not None
    si5.on_wait = list(si5.on_wait) + [
        mybir.SyncWait(
            sync_type="semaphore",
            id=oid,
            ant_name=oname,
            wait_mode="sem-ge-imm",
            wait_value=16,
            wait_reg=None,
        )
    ]


_STATE = {}


def _get_state():
    if not _STATE:
        _STATE["consts"] = _host_constants()
        _STATE["nc"] = _build_module()
    return _STATE


def _in_maps(prob_map, gt_map):
    st = _get_state()
    m1, ft2, rm16 = st["consts"]
    prob = np.asarray(prob_map, dtype=np.float32).reshape(H, H)
    gt = np.asarray(gt_map, dtype=np.float32).reshape(H, H)

    pack1 = np.concatenate(
        [
            np.ascontiguousarray(gt.T).astype(np.float16),
            m1,
            prob.astype(np.float16),
            ft2,
            rm16,
        ],
        axis=1,
    )
    return [{"pack1": np.ascontiguousarray(pack1)}]


def _run(prob_map, gt_map, trace=False, **spmd_kwargs):
    from concourse import bass_utils

    st = _get_state()
    in_maps = _in_maps(prob_map, gt_map)
    res = bass_utils.run_bass_kernel_spmd(
        st["nc"], in_maps, core_ids=[0], trace=trace, **spmd_kwargs,
    )
    value = np.float32(np.asarray(res.results[0]["out"]).ravel()[0])
    return value, res


def kernel(prob_map, gt_map):
    # Two executions, return the second: the first run on a cold device can
    # read semaphore state left dirty by previously-executed unrelated
    # NEFFs (pre-satisfying an internal wait); from the second run on the
    # program's own start-of-run sem clear guarantees clean entry state.
    _run(prob_map, gt_map, trace=False)
    value, _ = _run(prob_map, gt_map, trace=False)
    return np.asarray(value, dtype=np.float32)
